# revision 3
# baseline (speedup 1.0000x reference)
"""Trainium2 Bass kernel for nn_AutoregressiveResidualBlock (dense_cnn).

Reference computation (per batch row, eval-mode BN, dilated queues of len 1 used):
    l1      = interleave(q1, x)                  # (bs, 1024), q1 = conv1_queue[0]
    h1      = relu(l1 @ w1.T + b1)
    h1bn    = h1 * s1 + t1                       # BN1 folded: s1 = g1/sqrt(v1+eps)
    l2      = interleave(q2, h1bn)               # (bs, 2048), q2 = conv2_queue[0]
    pre2    = l2 @ w2.T + b2 + l1 @ w_skip.T + b_skip
    out     = relu(pre2) * s2 + t2               # BN2 folded

Device strategy (pure data-parallel over 8 cores, bs 16384 -> 2048/core):
  * interleave is eliminated by splitting every weight into even/odd column
    halves (even pairs with queue channels, odd with x / h1bn channels).
  * BN1 scale is folded into conv1's PSUM eviction:  h1s = relu(s1*psum + s1*b1)
    (valid since s1 > 0), BN1 shift t1 flows into the conv2 bias c2 on host.
  * all matmuls run channels-on-partitions (mapping out.T = W @ act.T) in
    float32r (full-rate PE; ~1e-4 scaled error vs fp32), activations are
    transposed on-chip via fp32 PE transposes (bit-exact), weights are
    pre-transposed/deinterleaved on host and DMA'd directly as f32r.
  * conv2 epilogue: z = relu(s2*psum + s2*c2) per out-channel partition, then
    PE-transpose back to batch-major, + t2 fused into the PSUM eviction add.
"""
import sys

sys.path.insert(0, "/opt/trn_rl_repo")

import numpy as np
import concourse.bass as bass
import concourse.mybir as mybir
from concourse.tile import TileContext
from concourse.bass_utils import run_bass_kernel_spmd
from concourse.masks import make_identity

P = 128
NCORES = 8
BS_FULL = 16384
BS = BS_FULL // NCORES   # 2048 rows per core
BLK = 512                # batch block (matmul moving free dim)
NB = BS // BLK           # 4
DIN = 512
MID = 1024
OUT = 512
KD = DIN // P            # 4  (x / q1 channel chunks)
KM = MID // P            # 8  (q2 / h1 channel chunks)
MT = MID // P            # 8  conv1 out tiles
OT = OUT // P            # 4  conv2 out tiles
BT = BLK // P            # 4  batch subtiles per block
EPS = 1e-5

f32 = mybir.dt.float32
f32r = mybir.dt.float32r
RELU = mybir.ActivationFunctionType.Relu
ACT_COPY = mybir.ActivationFunctionType.Copy
ADD = mybir.AluOpType.add

_nc_cache = [None]


# --------------------------------------------------------------------------
# wait-splitting post-pass: this container's walrus rejects >1 inline sem wait
# on several opcodes (Matmult: 1; CTRL NoOp/Drain: ~4).  Hoist excess waits
# onto same-engine NoOps inserted immediately before the instruction —
# semantically identical (the engine blocks at the NoOp instead).
_wfix_counter = [0]


def _fix_block_waits(b, cap, nop_cap):
    il = b.instructions
    i = 0
    while i < len(il):
        inst = il[i]
        body = getattr(inst, 'body_bb', None)
        if body is not None:
            _fix_block_waits(body, cap, nop_cap)
        si = inst.sync_info
        if si is None:
            i += 1
            continue
        w = list(si.on_wait or [])
        if len(w) <= cap:
            i += 1
            continue
        keep = w[-cap:]
        excess = w[:-cap]
        nops = []
        for j in range(0, len(excess), nop_cap):
            chunk = excess[j:j + nop_cap]
            _wfix_counter[0] += 1
            nop = mybir.InstNoOp(name=f"I-wfix-{_wfix_counter[0]}", ins=[], outs=[])
            nop.engine = inst.engine
            nop.sync_info = mybir.SyncInfo(on_wait=chunk, on_update=[])
            nops.append(nop)
        si.on_wait = keep
        inst.sync_info = si
        il[i:i] = nops
        i += len(nops) + 1


def fix_waits(nc, cap=1, nop_cap=1):
    for b in nc.m.functions[0].blocks:
        _fix_block_waits(b, cap, nop_cap)
    return nc


# --------------------------------------------------------------------------
def build_nc():
    nc = bass.Bass()
    x_d = nc.declare_dram_parameter("x", [BS, DIN], f32, isOutput=False)
    q1_d = nc.declare_dram_parameter("q1", [BS, DIN], f32, isOutput=False)
    q2_d = nc.declare_dram_parameter("q2", [BS, MID], f32, isOutput=False)
    w1eT_d = nc.declare_dram_parameter("w1eT", [DIN, MID], f32r, isOutput=False)
    w1oT_d = nc.declare_dram_parameter("w1oT", [DIN, MID], f32r, isOutput=False)
    w2eT_d = nc.declare_dram_parameter("w2eT", [MID, OUT], f32r, isOutput=False)
    w2oT_d = nc.declare_dram_parameter("w2oT", [MID, OUT], f32r, isOutput=False)
    wseT_d = nc.declare_dram_parameter("wseT", [DIN, OUT], f32r, isOutput=False)
    wsoT_d = nc.declare_dram_parameter("wsoT", [DIN, OUT], f32r, isOutput=False)
    s1v_d = nc.declare_dram_parameter("s1v", [P, MT], f32, isOutput=False)
    s1b1v_d = nc.declare_dram_parameter("s1b1v", [P, MT], f32, isOutput=False)
    s2v_d = nc.declare_dram_parameter("s2v", [P, OT], f32, isOutput=False)
    s2c2v_d = nc.declare_dram_parameter("s2c2v", [P, OT], f32, isOutput=False)
    t2rep_d = nc.declare_dram_parameter("t2rep", [P, OUT], f32, isOutput=False)
    out_d = nc.declare_dram_parameter("out", [BS, OUT], f32, isOutput=True)

    with TileContext(nc) as tc:
        with (
            tc.tile_pool(name="wpool", bufs=1) as wpool,
            tc.tile_pool(name="const", bufs=1) as const,
            tc.tile_pool(name="rawA", bufs=3) as rawA,
            tc.tile_pool(name="rawB", bufs=3) as rawB,
            tc.tile_pool(name="actp", bufs=1) as actp,
            tc.tile_pool(name="hpool", bufs=1) as hpool,
            tc.tile_pool(name="zpool", bufs=1) as zpool,
            tc.tile_pool(name="opool", bufs=2) as opool,
            tc.tile_pool(name="tpsum", bufs=4, space="PSUM") as tpsum,
            tc.tile_pool(name="mpsum", bufs=3, space="PSUM") as mpsum,
        ):
            # ---- resident weights (K-major, f32r straight from DRAM) ----
            w1e = []
            w1o = []
            for k in range(KD):
                t = wpool.tile([P, MID], f32r, tag=f"w1e{k}")
                nc.sync.dma_start(out=t[:], in_=w1eT_d[k * P:(k + 1) * P, :])
                w1e.append(t)
                t = wpool.tile([P, MID], f32r, tag=f"w1o{k}")
                nc.sync.dma_start(out=t[:], in_=w1oT_d[k * P:(k + 1) * P, :])
                w1o.append(t)
            w2e = []
            w2o = []
            for k in range(KM):
                t = wpool.tile([P, OUT], f32r, tag=f"w2e{k}")
                nc.sync.dma_start(out=t[:], in_=w2eT_d[k * P:(k + 1) * P, :])
                w2e.append(t)
                t = wpool.tile([P, OUT], f32r, tag=f"w2o{k}")
                nc.sync.dma_start(out=t[:], in_=w2oT_d[k * P:(k + 1) * P, :])
                w2o.append(t)
            wse = []
            wso = []
            for k in range(KD):
                t = wpool.tile([P, OUT], f32r, tag=f"wse{k}")
                nc.sync.dma_start(out=t[:], in_=wseT_d[k * P:(k + 1) * P, :])
                wse.append(t)
                t = wpool.tile([P, OUT], f32r, tag=f"wso{k}")
                nc.sync.dma_start(out=t[:], in_=wsoT_d[k * P:(k + 1) * P, :])
                wso.append(t)

            # ---- constants ----
            ident = const.tile([P, P], f32)
            make_identity(nc, ident[:])
            s1v = const.tile([P, MT], f32)
            nc.sync.dma_start(out=s1v[:], in_=s1v_d[:])
            s1b1v = const.tile([P, MT], f32)
            nc.sync.dma_start(out=s1b1v[:], in_=s1b1v_d[:])
            s2v = const.tile([P, OT], f32)
            nc.sync.dma_start(out=s2v[:], in_=s2v_d[:])
            s2c2v = const.tile([P, OT], f32)
            nc.sync.dma_start(out=s2c2v[:], in_=s2c2v_d[:])
            t2rep = const.tile([P, OUT], f32)
            nc.sync.dma_start(out=t2rep[:], in_=t2rep_d[:])

            # ---- main loop over batch blocks ----
            for b in range(NB):
                base = b * BLK
                xr, q1r, q2r = [], [], []
                for j in range(BT):
                    t = rawA.tile([P, DIN], f32, tag="xr")
                    nc.sync.dma_start(out=t[:], in_=x_d[base + j * P: base + (j + 1) * P, :])
                    xr.append(t)
                for j in range(BT):
                    t = rawA.tile([P, DIN], f32, tag="q1r")
                    nc.sync.dma_start(out=t[:], in_=q1_d[base + j * P: base + (j + 1) * P, :])
                    q1r.append(t)
                for j in range(BT):
                    t = rawB.tile([P, MID], f32, tag="q2r")
                    nc.sync.dma_start(out=t[:], in_=q2_d[base + j * P: base + (j + 1) * P, :])
                    q2r.append(t)

                # transpose to channels-on-partitions (fp32 PE transpose,
                # ACT eviction casts to f32r = the rounding the verifier wants)
                def transpose_in(raw_tiles, nchunks, tag):
                    # j outer so each raw tile is fully consumed (and its pool
                    # slot released) before the next one is needed
                    outs = [actp.tile([P, BLK], f32r, tag=f"{tag}{c}",
                                      name=f"{tag}{c}_{b}")
                            for c in range(nchunks)]
                    for j in range(BT):
                        for c in range(nchunks):
                            pst = tpsum.tile([P, P], f32, tag="tp")
                            nc.tensor.transpose(
                                pst[:], raw_tiles[j][:, c * P:(c + 1) * P], ident[:])
                            nc.scalar.activation(
                                outs[c][:, j * P:(j + 1) * P], pst[:], ACT_COPY)
                    return outs

                xT = transpose_in(xr, KD, "xT")
                q1T = transpose_in(q1r, KD, "q1T")
                q2T = transpose_in(q2r, KM, "q2T")

                # conv1: h1s[mid, bs] = relu(s1*(W1 l1T) + s1*b1)
                h1 = []
                for m in range(MT):
                    ps = mpsum.tile([P, BLK], f32, tag="mm")
                    for k in range(KD):
                        nc.tensor.matmul(ps[:], w1e[k][:, m * P:(m + 1) * P], q1T[k][:],
                                         start=(k == 0), stop=False)
                    for k in range(KD):
                        nc.tensor.matmul(ps[:], w1o[k][:, m * P:(m + 1) * P], xT[k][:],
                                         start=False, stop=(k == KD - 1))
                    ht = hpool.tile([P, BLK], f32r, tag=f"h1{m}")
                    nc.scalar.activation(ht[:], ps[:], RELU,
                                         scale=s1v[:, m:m + 1], bias=s1b1v[:, m:m + 1])
                    h1.append(ht)

                # conv2 + skip: z[out, bs] = relu(s2*pre2 + s2*c2)
                zs = []
                for o in range(OT):
                    ps = mpsum.tile([P, BLK], f32, tag="mm")
                    for k in range(KM):
                        nc.tensor.matmul(ps[:], w2e[k][:, o * P:(o + 1) * P], q2T[k][:],
                                         start=(k == 0), stop=False)
                    for k in range(KM):
                        nc.tensor.matmul(ps[:], w2o[k][:, o * P:(o + 1) * P], h1[k][:],
                                         start=False, stop=False)
                    for k in range(KD):
                        nc.tensor.matmul(ps[:], wse[k][:, o * P:(o + 1) * P], q1T[k][:],
                                         start=False, stop=False)
                    for k in range(KD):
                        nc.tensor.matmul(ps[:], wso[k][:, o * P:(o + 1) * P], xT[k][:],
                                         start=False, stop=(k == KD - 1))
                    zt = zpool.tile([P, BLK], f32, tag=f"z{o}")
                    nc.scalar.activation(zt[:], ps[:], RELU,
                                         scale=s2v[:, o:o + 1], bias=s2c2v[:, o:o + 1])
                    zs.append(zt)

                # transpose back to batch-major, fuse "+ t2" into eviction, store
                for j in range(BT):
                    ob = opool.tile([P, OUT], f32, tag=f"ob{j % 2}")
                    for o in range(OT):
                        pst = tpsum.tile([P, P], f32, tag="tp")
                        nc.tensor.transpose(pst[:], zs[o][:, j * P:(j + 1) * P], ident[:])
                        nc.vector.tensor_tensor(
                            out=ob[:, o * P:(o + 1) * P], in0=pst[:],
                            in1=t2rep[:, o * P:(o + 1) * P], op=ADD)
                    nc.sync.dma_start(
                        out=out_d[base + j * P: base + (j + 1) * P, :], in_=ob[:])
    fix_waits(nc)
    return nc


def _get_nc():
    if _nc_cache[0] is None:
        _nc_cache[0] = build_nc()
    return _nc_cache[0]


# --------------------------------------------------------------------------
def _host_prep(inputs):
    x = np.ascontiguousarray(inputs["x"][:, :, 0], dtype=np.float32)
    q1 = np.ascontiguousarray(inputs["conv1_queue"][0, :, :, 0], dtype=np.float32)
    q2 = np.ascontiguousarray(inputs["conv2_queue"][0, :, :, 0], dtype=np.float32)
    w1 = np.asarray(inputs["w1"], dtype=np.float32)
    w2 = np.asarray(inputs["w2"], dtype=np.float32)
    ws = np.asarray(inputs["w_skip"], dtype=np.float32)
    b1 = np.asarray(inputs["b1"], dtype=np.float32)
    b2 = np.asarray(inputs["b2"], dtype=np.float32)
    bsk = np.asarray(inputs["b_skip"], dtype=np.float32)

    s1 = (inputs["bn1_scale"] / np.sqrt(inputs["bn1_var"] + EPS)).astype(np.float32)
    t1 = (inputs["bn1_bias"] - inputs["bn1_mean"] * s1).astype(np.float32)
    s2 = (inputs["bn2_scale"] / np.sqrt(inputs["bn2_var"] + EPS)).astype(np.float32)
    t2 = (inputs["bn2_bias"] - inputs["bn2_mean"] * s2).astype(np.float32)
    w2o_raw = w2[:, 1::2]
    c2 = (b2 + w2o_raw @ t1 + bsk).astype(np.float32)

    def kmajor(w):  # (out, in) -> contiguous (in, out)
        return np.ascontiguousarray(w.T)

    rep = {
        "w1eT": kmajor(w1[:, 0::2]),
        "w1oT": kmajor(w1[:, 1::2]),
        "w2eT": kmajor(w2[:, 0::2]),
        "w2oT": kmajor(w2o_raw),
        "wseT": kmajor(ws[:, 0::2]),
        "wsoT": kmajor(ws[:, 1::2]),
        "s1v": np.ascontiguousarray(s1.reshape(MT, P).T),
        "s1b1v": np.ascontiguousarray((s1 * b1).reshape(MT, P).T),
        "s2v": np.ascontiguousarray(s2.reshape(OT, P).T),
        "s2c2v": np.ascontiguousarray((s2 * c2).reshape(OT, P).T),
        "t2rep": np.ascontiguousarray(np.broadcast_to(t2, (P, OUT))),
    }
    in_maps = []
    for i in range(NCORES):
        sl = slice(i * BS, (i + 1) * BS)
        m = {"x": x[sl], "q1": q1[sl], "q2": q2[sl]}
        m.update(rep)
        in_maps.append(m)
    return in_maps


def _run(inputs, trace=False, **trace_kw):
    in_maps = _host_prep(inputs)
    nc = _get_nc()
    res = run_bass_kernel_spmd(nc, in_maps, list(range(NCORES)), trace=trace,
                               **trace_kw)
    out = np.concatenate([r["out"] for r in res.results], axis=0)
    return out[:, :, None].astype(np.float32), res


def kernel(**inputs) -> np.ndarray:
    out, _ = _run(inputs, trace=False)
    return out


# revision 24
# speedup vs baseline: 21.2978x; 21.2978x over previous
"""Trainium2 Bass kernel for nn_AutoregressiveResidualBlock (dense_cnn).

Reference computation (per batch row, eval-mode BN, dilated queues of len 1 used):
    l1      = interleave(q1, x)                  # (bs, 1024), q1 = conv1_queue[0]
    h1      = relu(l1 @ w1.T + b1)
    h1bn    = h1 * s1 + t1                       # BN1 folded: s1 = g1/sqrt(v1+eps)
    l2      = interleave(q2, h1bn)               # (bs, 2048), q2 = conv2_queue[0]
    pre2    = l2 @ w2.T + b2 + l1 @ w_skip.T + b_skip
    out     = relu(pre2) * s2 + t2               # BN2 folded

Device strategy (pure data-parallel over 8 cores, bs 16384 -> 2048/core):
  * interleave is eliminated by splitting every weight into even/odd column
    halves (even pairs with queue channels, odd with x / h1bn channels).
  * BN1 scale is folded into conv1's PSUM eviction:  h1s = relu(s1*psum + s1*b1)
    (valid since s1 > 0), BN1 shift t1 flows into the conv2 bias c2 on host.
  * all matmuls run channels-on-partitions (mapping out.T = W @ act.T) in
    float32r (full-rate PE; ~1e-4 scaled error vs fp32), activations are
    transposed on-chip via fp32 PE transposes (bit-exact), weights are
    pre-transposed/deinterleaved on host and DMA'd directly as f32r.
  * conv2 runs batch-major (activations stationary, weights moving), so the
    output needs no transpose: weights carry the BN2 scale s2, the bias
    s2*c2 enters as a K=1 ones-row matmul, relu happens on the ACT eviction
    and "+t2" is fused into the DVE store-side add.
  * DMA lanes: weights via gpsimd/SWDGE (Pool), input tiles via SP, output
    stores + small consts via ACT -- keeps every lane off the critical path.
"""
import sys

sys.path.insert(0, "/opt/trn_rl_repo")

import numpy as np
import concourse.bass as bass
import concourse.mybir as mybir
from concourse.tile import TileContext
from concourse.bass_utils import run_bass_kernel_spmd
from concourse.masks import make_identity

P = 128
NCORES = 8
BS_FULL = 16384
BS = BS_FULL // NCORES   # 2048 rows per core
BLK = 512                # batch block (matmul moving free dim)
NB = BS // BLK           # 4
DIN = 512
MID = 1024
OUT = 512
KD = DIN // P            # 4  (x / q1 channel chunks)
KM = MID // P            # 8  (q2 / h1 channel chunks)
MT = MID // P            # 8  conv1 out tiles
OT = OUT // P            # 4  conv2 out tiles
BT = BLK // P            # 4  batch subtiles per block
EPS = 1e-5

f32 = mybir.dt.float32
f32r = mybir.dt.float32r
RELU = mybir.ActivationFunctionType.Relu
ACT_COPY = mybir.ActivationFunctionType.Copy
ADD = mybir.AluOpType.add

_nc_cache = [None]


# --------------------------------------------------------------------------
# wait-splitting post-pass: this container's walrus rejects >1 inline sem wait
# on several opcodes (Matmult: 1; CTRL NoOp/Drain: ~4).  Hoist excess waits
# onto same-engine NoOps inserted immediately before the instruction —
# semantically identical (the engine blocks at the NoOp instead).
_wfix_counter = [0]


def _fix_block_waits(b, cap, nop_cap):
    il = b.instructions
    i = 0
    while i < len(il):
        inst = il[i]
        body = getattr(inst, 'body_bb', None)
        if body is not None:
            _fix_block_waits(body, cap, nop_cap)
        si = inst.sync_info
        if si is None:
            i += 1
            continue
        w = list(si.on_wait or [])
        if len(w) <= cap:
            i += 1
            continue
        keep = w[-cap:]
        excess = w[:-cap]
        nops = []
        for j in range(0, len(excess), nop_cap):
            chunk = excess[j:j + nop_cap]
            _wfix_counter[0] += 1
            nop = mybir.InstNoOp(name=f"I-wfix-{_wfix_counter[0]}", ins=[], outs=[])
            nop.engine = inst.engine
            nop.sync_info = mybir.SyncInfo(on_wait=chunk, on_update=[])
            nops.append(nop)
        si.on_wait = keep
        inst.sync_info = si
        il[i:i] = nops
        i += len(nops) + 1


def fix_waits(nc, cap=1, nop_cap=1):
    for b in nc.m.functions[0].blocks:
        _fix_block_waits(b, cap, nop_cap)
    return nc


# --------------------------------------------------------------------------
def build_nc():
    nc = bass.Bass()
    x_d = nc.declare_dram_parameter("x", [BS, DIN], f32r, isOutput=False)
    q1_d = nc.declare_dram_parameter("q1", [BS, DIN], f32r, isOutput=False)
    q2_d = nc.declare_dram_parameter("q2", [BS, MID], f32r, isOutput=False)
    w1eT_d = nc.declare_dram_parameter("w1eT", [DIN, MID], f32r, isOutput=False)
    w1oT_d = nc.declare_dram_parameter("w1oT", [DIN, MID], f32r, isOutput=False)
    w2eT_d = nc.declare_dram_parameter("w2eT", [MID, OUT], f32r, isOutput=False)
    w2oT_d = nc.declare_dram_parameter("w2oT", [MID, OUT], f32r, isOutput=False)
    wseT_d = nc.declare_dram_parameter("wseT", [DIN, OUT], f32r, isOutput=False)
    wsoT_d = nc.declare_dram_parameter("wsoT", [DIN, OUT], f32r, isOutput=False)
    s1v_d = nc.declare_dram_parameter("s1v", [P, MT], f32, isOutput=False)
    s1b1v_d = nc.declare_dram_parameter("s1b1v", [P, MT], f32, isOutput=False)
    s2c2r_d = nc.declare_dram_parameter("s2c2r", [1, OUT], f32r, isOutput=False)
    t2rep_d = nc.declare_dram_parameter("t2rep", [P, OUT], f32, isOutput=False)
    out_d = nc.declare_dram_parameter("out", [BS, OUT], f32, isOutput=True)

    with TileContext(nc) as tc:
        with (
            tc.tile_pool(name="wpool", bufs=1) as wpool,
            tc.tile_pool(name="const", bufs=1) as const,
            tc.tile_pool(name="rawA", bufs=4) as rawA,
            tc.tile_pool(name="rawB", bufs=2) as rawB,
            tc.tile_pool(name="actp", bufs=1) as actp,
            tc.tile_pool(name="hpool", bufs=1) as hpool,
            tc.tile_pool(name="zpool", bufs=1) as zpool,
            tc.tile_pool(name="opool", bufs=2) as opool,
            tc.tile_pool(name="tpsum", bufs=5, space="PSUM") as tpsum,
            tc.tile_pool(name="mpsum", bufs=3, space="PSUM") as mpsum,
        ):
            # ---- block-0 x/q1 raw tiles first so PE can start immediately --
            pre_xr, pre_q1r = [], []
            for j in range(BT):
                t = rawA.tile([P, DIN], f32r, tag="xr", name=f"xr_pre{j}")
                nc.sync.dma_start(out=t[:], in_=x_d[j * P:(j + 1) * P, :])
                pre_xr.append(t)
            for j in range(BT):
                t = rawA.tile([P, DIN], f32r, tag="q1r", name=f"q1r_pre{j}")
                nc.sync.dma_start(out=t[:], in_=q1_d[j * P:(j + 1) * P, :])
                pre_q1r.append(t)

            # ---- constants ----
            identf = const.tile([P, P], f32)
            make_identity(nc, identf[:])
            ident = const.tile([P, P], f32r)
            nc.vector.tensor_copy(out=ident[:], in_=identf[:])
            s1v = const.tile([P, MT], f32)
            nc.scalar.dma_start(out=s1v[:], in_=s1v_d[:])
            s1b1v = const.tile([P, MT], f32)
            nc.scalar.dma_start(out=s1b1v[:], in_=s1b1v_d[:])
            s2c2r = const.tile([1, OUT], f32r)
            nc.scalar.dma_start(out=s2c2r[:], in_=s2c2r_d[:])
            onesr = const.tile([1, P], f32)
            nc.gpsimd.memset(onesr[:], 1.0)
            ones = const.tile([1, P], f32r)
            nc.vector.tensor_copy(out=ones[:], in_=onesr[:])
            t2rep = const.tile([P, OUT], f32)
            nc.scalar.dma_start(out=t2rep[:], in_=t2rep_d[:])

            # ---- resident weights (K-major, f32r straight from DRAM) ----
            # w1 on the SP lane (needed first), w2/skip/consts on ACT's lane
            w1e = []
            w1o = []
            for k in range(KD):
                t = wpool.tile([P, MID], f32r, tag=f"w1o{k}")
                nc.gpsimd.dma_start(out=t[:], in_=w1oT_d[k * P:(k + 1) * P, :])
                w1o.append(t)
            for k in range(KD):
                t = wpool.tile([P, MID], f32r, tag=f"w1e{k}")
                nc.gpsimd.dma_start(out=t[:], in_=w1eT_d[k * P:(k + 1) * P, :])
                w1e.append(t)
            w2e = []
            w2o = []
            for k in range(KM):
                t = wpool.tile([P, OUT], f32r, tag=f"w2e{k}")
                nc.gpsimd.dma_start(out=t[:], in_=w2eT_d[k * P:(k + 1) * P, :])
                w2e.append(t)
                t = wpool.tile([P, OUT], f32r, tag=f"w2o{k}")
                nc.gpsimd.dma_start(out=t[:], in_=w2oT_d[k * P:(k + 1) * P, :])
                w2o.append(t)
            wse = []
            wso = []
            for k in range(KD):
                t = wpool.tile([P, OUT], f32r, tag=f"wse{k}")
                nc.gpsimd.dma_start(out=t[:], in_=wseT_d[k * P:(k + 1) * P, :])
                wse.append(t)
                t = wpool.tile([P, OUT], f32r, tag=f"wso{k}")
                nc.gpsimd.dma_start(out=t[:], in_=wsoT_d[k * P:(k + 1) * P, :])
                wso.append(t)

            # ---- main loop over batch blocks ----
            for b in range(NB):
                base = b * BLK
                xr, q1r, q2r = [], [], []
                if b == 0:
                    xr, q1r = pre_xr, pre_q1r
                else:
                    for j in range(BT):
                        t = rawA.tile([P, DIN], f32r, tag="xr")
                        nc.sync.dma_start(out=t[:], in_=x_d[base + j * P: base + (j + 1) * P, :])
                        xr.append(t)
                    for j in range(BT):
                        t = rawA.tile([P, DIN], f32r, tag="q1r")
                        nc.sync.dma_start(out=t[:], in_=q1_d[base + j * P: base + (j + 1) * P, :])
                        q1r.append(t)
                for j in range(BT):
                    t = rawB.tile([P, MID], f32r, tag="q2r")
                    nc.sync.dma_start(out=t[:], in_=q2_d[base + j * P: base + (j + 1) * P, :])
                    q2r.append(t)

                # transpose to channels-on-partitions (fp32 PE transpose,
                # ACT eviction casts to f32r = the rounding the verifier wants)
                # Transpose phase: per raw tile j, all chunk transposes land
                # in ONE [P, nchunks*P] psum tile (whole bank), evicted with a
                # single wide copy (alternating DVE/ACT) into a wide
                # channels-major tile laid out [P, nchunks*BLK]:
                #   wide[:, c*BLK + j*P : c*BLK + (j+1)*P] = chunk c of row j
                def transpose_j(raw_tiles, wide, nchunks, j, tag):
                    pst = tpsum.tile([P, nchunks * P], f32r, tag="tp",
                                     name=f"t{tag}_{b}_{j}")
                    for c in range(nchunks):
                        nc.tensor.transpose(
                            pst[:, c * P:(c + 1) * P],
                            raw_tiles[j][:, c * P:(c + 1) * P], ident[:])
                    src = pst[:].rearrange("p (c w) -> p c w", c=nchunks)
                    dst = wide[:].rearrange("p (c v) -> p c v", c=nchunks)[
                        :, :, j * P:(j + 1) * P]
                    if j % 2 == 0:
                        nc.vector.tensor_copy(out=dst, in_=src)
                    else:
                        nc.scalar.activation(dst, src, ACT_COPY)

                xTw = actp.tile([P, KD * BLK], f32r, tag="xTw", name=f"xTw_{b}")
                q1Tw = actp.tile([P, KD * BLK], f32r, tag="q1Tw", name=f"q1Tw_{b}")
                for j in range(BT):
                    transpose_j(xr, xTw, KD, j, "x")
                for j in range(BT):
                    transpose_j(q1r, q1Tw, KD, j, "q1")
                xT = [xTw[:, c * BLK:(c + 1) * BLK] for c in range(KD)]
                q1T = [q1Tw[:, c * BLK:(c + 1) * BLK] for c in range(KD)]

                # q2 transposes interleave into conv1's m-loop (2 psum-batches
                # of 4 chunks per raw tile j -> 8 batches); per-j tiles so
                # conv2 group j only depends on its own evictions
                q2Tj = [actp.tile([P, KM * P], f32r, tag=f"q2Tj{j}",
                                  name=f"q2Tj{j}_{b}") for j in range(BT)]
                q2_batches = [(j, h) for j in range(BT) for h in range(2)]

                def emit_q2_transposes(n):
                    for _ in range(n):
                        if not q2_batches:
                            return
                        j, h = q2_batches.pop(0)
                        pst = tpsum.tile([P, KD * P], f32r, tag="tp",
                                         name=f"tq2_{b}_{j}_{h}")
                        for ci in range(KD):
                            c = h * KD + ci
                            nc.tensor.transpose(
                                pst[:, ci * P:(ci + 1) * P],
                                q2r[j][:, c * P:(c + 1) * P], ident[:])
                        src = pst[:].rearrange("p (c w) -> p c w", c=KD)
                        dst = q2Tj[j][:].rearrange("p (c w) -> p c w", c=KM)[
                            :, h * KD:(h + 1) * KD, :]
                        if (j + h) % 2 == 0:
                            nc.vector.tensor_copy(out=dst, in_=src)
                        else:
                            nc.scalar.activation(dst, src, ACT_COPY)

                # conv1: h1s[mid, bs] = relu(s1*(W1 l1T) + s1*b1)
                h1 = []
                for m in range(MT):
                    ps = mpsum.tile([P, BLK], f32, tag="mm")
                    for k in range(KD):
                        nc.tensor.matmul(ps[:], w1o[k][:, m * P:(m + 1) * P], xT[k][:],
                                         start=(k == 0), stop=False)
                    for k in range(KD):
                        nc.tensor.matmul(ps[:], w1e[k][:, m * P:(m + 1) * P], q1T[k][:],
                                         start=False, stop=(k == KD - 1))
                    ht = hpool.tile([P, BLK], f32r, tag=f"h1{m}")
                    nc.scalar.activation(ht[:], ps[:], RELU,
                                         scale=s1v[:, m:m + 1], bias=s1b1v[:, m:m + 1])
                    h1.append(ht)
                    emit_q2_transposes(1)
                emit_q2_transposes(len(q2_batches))

                # conv2 + skip, batch-major output:
                #   psum[bs_j, out] = s2*pre2 + s2*c2  (weights carry s2; bias
                #   via a K=1 ones-row matmul), then relu on ACT eviction and
                #   "+t2" fused into the DVE store-side add.
                for j in range(BT):
                    ps = mpsum.tile([P, OUT], f32, tag="mm")
                    nc.tensor.matmul(ps[:], ones[:], s2c2r[:],
                                     start=True, stop=False)
                    for k in range(KM):
                        nc.tensor.matmul(ps[:], q2Tj[j][:, k * P:(k + 1) * P],
                                         w2e[k][:], start=False, stop=False)
                    for k in range(KM):
                        nc.tensor.matmul(ps[:], h1[k][:, j * P:(j + 1) * P],
                                         w2o[k][:], start=False, stop=False)
                    for k in range(KD):
                        nc.tensor.matmul(ps[:], q1T[k][:, j * P:(j + 1) * P],
                                         wse[k][:], start=False, stop=False)
                    for k in range(KD):
                        nc.tensor.matmul(ps[:], xT[k][:, j * P:(j + 1) * P],
                                         wso[k][:], start=False, stop=(k == KD - 1))
                    zb = zpool.tile([P, OUT], f32, tag=f"zb{j % 2}",
                                    name=f"zb{b}_{j}")
                    nc.scalar.activation(zb[:], ps[:], RELU)
                    ob = opool.tile([P, OUT], f32, tag=f"ob{j % 2}",
                                    name=f"ob{b}_{j}")
                    nc.vector.tensor_tensor(out=ob[:], in0=zb[:],
                                            in1=t2rep[:], op=ADD)
                    nc.scalar.dma_start(
                        out=out_d[base + j * P: base + (j + 1) * P, :], in_=ob[:])
    fix_waits(nc)
    return nc


def _get_nc():
    if _nc_cache[0] is None:
        _nc_cache[0] = build_nc()
    return _nc_cache[0]


# --------------------------------------------------------------------------
def _host_prep(inputs):
    x = np.ascontiguousarray(inputs["x"][:, :, 0], dtype=np.float32)
    q1 = np.ascontiguousarray(inputs["conv1_queue"][0, :, :, 0], dtype=np.float32)
    q2 = np.ascontiguousarray(inputs["conv2_queue"][0, :, :, 0], dtype=np.float32)
    w1 = np.asarray(inputs["w1"], dtype=np.float32)
    w2 = np.asarray(inputs["w2"], dtype=np.float32)
    ws = np.asarray(inputs["w_skip"], dtype=np.float32)
    b1 = np.asarray(inputs["b1"], dtype=np.float32)
    b2 = np.asarray(inputs["b2"], dtype=np.float32)
    bsk = np.asarray(inputs["b_skip"], dtype=np.float32)

    s1 = (inputs["bn1_scale"] / np.sqrt(inputs["bn1_var"] + EPS)).astype(np.float32)
    t1 = (inputs["bn1_bias"] - inputs["bn1_mean"] * s1).astype(np.float32)
    s2 = (inputs["bn2_scale"] / np.sqrt(inputs["bn2_var"] + EPS)).astype(np.float32)
    t2 = (inputs["bn2_bias"] - inputs["bn2_mean"] * s2).astype(np.float32)
    w2o_raw = w2[:, 1::2]
    c2 = (b2 + w2o_raw @ t1 + bsk).astype(np.float32)

    def kmajor(w):  # (out, in) -> contiguous (in, out)
        return np.ascontiguousarray(w.T)

    # conv2/skip weights carry the BN2 scale (columns of the K-major layout)
    rep = {
        "w1eT": kmajor(w1[:, 0::2]),
        "w1oT": kmajor(w1[:, 1::2]),
        "w2eT": kmajor(w2[:, 0::2] * s2[:, None]),
        "w2oT": kmajor(w2o_raw * s2[:, None]),
        "wseT": kmajor(ws[:, 0::2] * s2[:, None]),
        "wsoT": kmajor(ws[:, 1::2] * s2[:, None]),
        "s1v": np.ascontiguousarray(s1.reshape(MT, P).T),
        "s1b1v": np.ascontiguousarray((s1 * b1).reshape(MT, P).T),
        "s2c2r": np.ascontiguousarray((s2 * c2).reshape(1, OUT)),
        "t2rep": np.ascontiguousarray(np.broadcast_to(t2, (P, OUT))),
    }
    in_maps = []
    for i in range(NCORES):
        sl = slice(i * BS, (i + 1) * BS)
        m = {"x": x[sl], "q1": q1[sl], "q2": q2[sl]}
        m.update(rep)
        in_maps.append(m)
    return in_maps


def _run(inputs, trace=False, **trace_kw):
    in_maps = _host_prep(inputs)
    nc = _get_nc()
    res = run_bass_kernel_spmd(nc, in_maps, list(range(NCORES)), trace=trace,
                               **trace_kw)
    out = np.concatenate([r["out"] for r in res.results], axis=0)
    return out[:, :, None].astype(np.float32), res


def kernel(**inputs) -> np.ndarray:
    out, _ = _run(inputs, trace=False)
    return out
